# revision 30
# baseline (speedup 1.0000x reference)
"""GCN AutoEncoder on 8 Trainium2 NeuronCores (Bass/Tile) — v2.

The kernel is bound by SWDGE descriptor generation on the GpSimd Q7
(~2.4ns per gathered edge).  v2 keeps the Q7 busy generating descriptors
from t~0 to the end of the kernel:

  - PREPARE_ONLY gather pipeline: dma_gather(prepare_only=True) writes the
    descriptor ring ahead of data availability; trigger_dma fires them once
    the AllGather'd table lands.  Desc-gen for layer 1 runs under phase A +
    AllGather-1; desc-gen for layer 2 runs under the tail of layer 1 and
    AllGather-2.  (Tile's auto-wiring of consumer waits for prepared
    gathers targets DMASW lane sems nothing increments; a post-tile pass
    rewires those waits to the per-queue descriptor-baked sems.)
  - ONE row map for both layers: x is resharded by the node permutation on
    the host, so phase A computes each core's aggregation-dst rows
    directly, table1 and table2 share the permuted layout, and both layers
    share a single index tensor (and the self-loop term is added from
    SBUF-resident local rows instead of being gathered: ~5% fewer
    descriptors, and the L2 class-matching padding penalty disappears).
  - int16 gather indices cannot span the 51024-row table, so edges split
    into lo/hi window buckets; per-tile-group minimax slot counts
    max(deg, lb_max+hb_max) with nodes tiled by (deg, lb, hb) within
    row-class and high-out-degree nodes pinned to flex rows.
"""

import os

import ml_dtypes
import numpy as np

import concourse.bass as bass
import concourse.bacc as bacc
import concourse.mybir as mybir
import concourse.tile as tile
from concourse.bass_utils import run_bass_kernel_spmd

F32 = mybir.dt.float32
BF16 = mybir.dt.bfloat16
I16 = mybir.dt.int16

IN = 512
H = 64
L = 32
C = 8   # cores
NQ = 4  # SWDGE queues
GCAP = 40   # max slots per gather (ring: 4 gathers/queue * 40*8 descs < 1536)
CH = 13     # phase-A tiles per xT load chunk
SCRATCH = 24576  # SWDGE descriptor-ring carveout bytes/queue (1536 descs)

LAST_RESULTS = None

ZPAD = 128  # zero rows per shard (padding gathers spread across them)


class Sizes:
    def __init__(self, n):
        self.N = n
        self.NPC = n // C
        assert self.NPC * C == n
        self.SROWS = self.NPC + ZPAD
        self.TROWS = self.SROWS * C
        self.HI_BASE = max(0, self.TROWS - 32768)
        assert self.TROWS - self.HI_BASE <= 32768 and self.NPC < 32768
        self.LO_PAD = self.NPC             # shard 0 zero-row base (lo window)
        s = 0
        while s * self.SROWS + self.NPC < self.HI_BASE:
            s += 1
        self.HI_PAD = s * self.SROWS + self.NPC - self.HI_BASE
        assert self.HI_PAD + ZPAD <= 32768
        self.NT = (self.NPC + 127) // 128
        self.TSZ = [128] * (self.NT - 1) + [self.NPC - (self.NT - 1) * 128]

    def row_of(self, c, r):
        return c * self.SROWS + r


# ---------------------------------------------------------------- host side

def _wrap_idx(arr_k128):
    flat = arr_k128.reshape(-1)
    w16 = flat.reshape(-1, 16).T
    return np.tile(w16, (8, 1)).astype(np.int16)


def _preprocess(sz, edge_index):
    n = sz.N
    src = np.asarray(edge_index[0], dtype=np.int64)
    dst = np.asarray(edge_index[1], dtype=np.int64)
    degg = np.bincount(dst, minlength=n).astype(np.int64)   # gather degree
    deg = degg + 1                                          # + self loop
    dis = (1.0 / np.sqrt(deg.astype(np.float64))).astype(np.float32)
    odeg = np.bincount(src, minlength=n).astype(np.int64)

    order = np.argsort(dst, kind="stable")
    srcs_sorted = src[order]
    indptr = np.zeros(n + 1, dtype=np.int64)
    np.cumsum(np.bincount(dst, minlength=n), out=indptr[1:])

    def slot_cls(t):
        tsz = sz.TSZ[t]
        slot_core = np.repeat(np.arange(C), tsz)
        slot_lane = np.tile(np.arange(tsz), C)
        srow = sz.row_of(slot_core, t * 128 + slot_lane)
        return (slot_core, slot_lane,
                np.where(srow < sz.HI_BASE, 0, np.where(srow < 32768, 1, 2)))

    # ---- pass 1: deg-sorted tentative tiles; flex rows take high out-degree
    pord = np.lexsort((-odeg, (-degg) // 3))
    cls_node = np.empty(n, dtype=np.int64)
    off = 0
    for t in range(sz.NT):
        tsz = sz.TSZ[t]
        g = pord[off: off + tsz * C]
        off += tsz * C
        _, _, scls = slot_cls(t)
        byo = g[np.argsort(-odeg[g], kind="stable")]
        nflex = int((scls == 1).sum())
        cls_node[byo[:nflex]] = 1
        rest = byo[nflex:]
        nlo = int((scls == 0).sum())
        cls_node[rest[:nlo]] = 0
        cls_node[rest[nlo:]] = 2

    lb = np.bincount(dst[cls_node[src] == 0], minlength=n)
    hb = np.bincount(dst[cls_node[src] == 2], minlength=n)

    # ---- pass 2: class-preserving regroup by (deg, lb, hb)
    perm_nodes = [np.empty(sz.NPC, dtype=np.int64) for _ in range(C)]
    row2 = np.empty(n, dtype=np.int64)
    cl_nodes = []
    cl_pos = []
    for cl in (0, 1, 2):
        nodes_cl = np.nonzero(cls_node == cl)[0]
        key = np.lexsort((-hb[nodes_cl], -lb[nodes_cl], (-degg[nodes_cl]) // 3))
        cl_nodes.append(nodes_cl[key])
        cl_pos.append(0)
    for t in range(sz.NT):
        tsz = sz.TSZ[t]
        slot_core, slot_lane, scls = slot_cls(t)
        for cl in (0, 1, 2):
            si = np.nonzero(scls == cl)[0]
            k = len(si)
            nd = cl_nodes[cl][cl_pos[cl]: cl_pos[cl] + k]
            cl_pos[cl] += k
            for j in range(k):
                c_, l_ = slot_core[si[j]], slot_lane[si[j]]
                perm_nodes[c_][t * 128 + l_] = nd[j]
                row2[nd[j]] = sz.row_of(c_, t * 128 + l_)

    # ---- per-tile minimax K
    K_lo, K_hi = [], []
    n_lo = np.zeros(n, dtype=np.int64)
    for t in range(sz.NT):
        g = np.concatenate(
            [perm_nodes[c][t * 128: t * 128 + sz.TSZ[t]] for c in range(C)])
        dmax = int(degg[g].max())
        lbm = int(lb[g].max())
        hbm = int(hb[g].max())
        klo = max(lbm, dmax - hbm)
        khi = max(hbm, dmax - klo)
        klo, khi = max(2, klo), max(2, khi)
        K_lo.append(klo)
        K_hi.append(khi)
        n_lo[g] = np.clip(degg[g] - khi, lb[g],
                          np.minimum(degg[g] - hb[g], klo))

    # ---- pack tiles into gather groups (lo and hi each capped at GCAP)
    groups = []
    cur = []
    slo = shi = 0
    for t in range(sz.NT):
        if cur and (slo + K_lo[t] > GCAP or shi + K_hi[t] > GCAP):
            groups.append(cur)
            cur, slo, shi = [], 0, 0
        cur.append(t)
        slo += K_lo[t]
        shi += K_hi[t]
    if cur:
        groups.append(cur)

    # ---- per-dst class-sorted src rows
    rows_all = row2[srcs_sorted]
    cls_all = np.where(rows_all < sz.HI_BASE, 0,
                       np.where(rows_all < 32768, 1, 2))
    seg_id = np.repeat(np.arange(n), degg)
    order2 = np.lexsort((cls_all, seg_id))
    rows_s = rows_all[order2]

    def tile_block(c, t, half, K, pad):
        nodes = perm_nodes[c][t * 128: t * 128 + sz.TSZ[t]]
        spread = (np.arange(128)[:, None] * K
                  + np.arange(K)[None, :]) % ZPAD
        arr = pad + spread.astype(np.int64)
        vals = []
        cnts = np.zeros(128, dtype=np.int64)
        for j, n_ in enumerate(nodes):
            a = indptr[n_]
            if half == 0:
                v = rows_s[a: a + n_lo[n_]]
            else:
                v = rows_s[a + n_lo[n_]: a + degg[n_]] - sz.HI_BASE
            vals.append(v)
            cnts[j] = len(v)
        mask = np.arange(K)[None, :] < cnts[:, None]
        arr[mask] = np.concatenate(vals)
        return _wrap_idx(arr.T.copy())

    idx_tensors = []
    for c in range(C):
        blocks = []
        for grp in groups:
            for t in grp:
                blocks.append(tile_block(c, t, 0, K_lo[t], sz.LO_PAD))
            for t in grp:
                blocks.append(tile_block(c, t, 1, K_hi[t], sz.HI_PAD))
        idx_tensors.append(np.concatenate(blocks, axis=1))

    disp = np.zeros((C, 128, sz.NT), dtype=np.float32)
    for c in range(C):
        for t in range(sz.NT):
            disp[c, : sz.TSZ[t], t] = dis[perm_nodes[c][t * 128:
                                                        t * 128 + sz.TSZ[t]]]

    return dict(perm_nodes=perm_nodes, Klo=K_lo, Khi=K_hi, idx=idx_tensors,
                disp=disp, groups=groups)


# -------------------------------------------------------------- device side

def _plan_cols(groups, Klo, Khi):
    res = []
    col = 0
    for grp in groups:
        slo = sum(Klo[t] for t in grp)
        shi = sum(Khi[t] for t in grp)
        lo_off, hi_off = col, col + slo * 8
        col += (slo + shi) * 8
        klo_off, khi_off = {}, {}
        o = 0
        for t in grp:
            klo_off[t] = o
            o += Klo[t]
        o = 0
        for t in grp:
            khi_off[t] = o
            o += Khi[t]
        res.append((grp, lo_off, hi_off, klo_off, khi_off, slo, shi))
    return res, col


def build_program(nc, sz, meta):
    NPC, NT, TSZ = sz.NPC, sz.NT, sz.TSZ
    Klo, Khi = meta["Klo"], meta["Khi"]
    plan, CC = _plan_cols(meta["groups"], Klo, Khi)

    xT_d = nc.dram_tensor("xT", [IN, NPC], BF16, kind="ExternalInput")
    idx_d = nc.dram_tensor("idx", [128, CC], I16, kind="ExternalInput")
    disp_d = nc.dram_tensor("disp", [128, NT], F32, kind="ExternalInput")
    w1 = nc.dram_tensor("w1", [IN, H], BF16, kind="ExternalInput")
    b1bc_d = nc.dram_tensor("b1bc", [128, H], F32, kind="ExternalInput")
    w2 = nc.dram_tensor("w2", [H, L], F32, kind="ExternalInput")
    b2_d = nc.dram_tensor("b2", [L, 1], F32, kind="ExternalInput")
    wd1 = nc.dram_tensor("wd1", [L, H], F32, kind="ExternalInput")
    bd1_d = nc.dram_tensor("bd1", [H, 1], F32, kind="ExternalInput")
    wd2 = nc.dram_tensor("wd2", [H, IN], F32, kind="ExternalInput")
    bd2_d = nc.dram_tensor("bd2", [128, 4], F32, kind="ExternalInput")
    ident_d = nc.dram_tensor("ident", [128, 128], F32, kind="ExternalInput")
    out = nc.dram_tensor("out", [NPC, IN], F32, kind="ExternalOutput")

    ACT = mybir.ActivationFunctionType
    ADD = mybir.AluOpType.add
    MULT = mybir.AluOpType.mult
    rg = [list(range(C))]

    qsems = [nc.alloc_semaphore(f"gsem{q}") for q in range(NQ)]
    state = dict(gctr=0, qpend=[[] for _ in range(NQ)], prep_pos=[],
                 trig_map={})

    with tile.TileContext(nc) as tc:
        with (
            tc.tile_pool(name="const", bufs=1) as cpool,
            tc.tile_pool(name="gat", bufs=8) as gpool,
            tc.tile_pool(name="upool", bufs=4) as upool,
            tc.tile_pool(name="epool", bufs=3) as epool,
            tc.tile_pool(name="opool", bufs=2) as opool,
            tc.tile_pool(name="ps8", bufs=2, space="PSUM") as pp8,
            tc.tile_pool(name="pss", bufs=2, space="PSUM") as pps,
            tc.tile_pool(name="pso", bufs=2, space="PSUM") as ppo,
            tc.tile_pool(name="psh", bufs=2, space="PSUM") as pph,
            tc.tile_pool(name="dram", bufs=1, space="DRAM") as dpool,
        ):
            # ---------------- constants
            w1_sb = cpool.tile([128, 4, H], BF16)
            nc.sync.dma_start(w1_sb[:], w1.ap().rearrange("(b p) f -> p b f", p=128))
            ident = cpool.tile([128, 128], F32)
            nc.sync.dma_start(ident[:], ident_d.ap())
            ident_bf = cpool.tile([128, 128], BF16)
            nc.vector.tensor_copy(ident_bf[:], ident[:])
            b1bc = cpool.tile([128, H], F32)
            nc.sync.dma_start(b1bc[:], b1bc_d.ap())
            w2_sb = cpool.tile([H, L], F32)
            nc.sync.dma_start(w2_sb[:], w2.ap())
            b2_sb = cpool.tile([L, 1], F32)
            nc.sync.dma_start(b2_sb[:], b2_d.ap())
            wd1_sb = cpool.tile([L, H], F32)
            nc.sync.dma_start(wd1_sb[:], wd1.ap())
            bd1_sb = cpool.tile([H, 1], F32)
            nc.sync.dma_start(bd1_sb[:], bd1_d.ap())
            wd2_sb = cpool.tile([H, IN], F32)
            nc.sync.dma_start(wd2_sb[:], wd2.ap())
            bd2_sb = cpool.tile([128, 4], F32)
            nc.sync.dma_start(bd2_sb[:], bd2_d.ap())
            disp = cpool.tile([128, NT], F32)
            nc.sync.dma_start(disp[:], disp_d.ap())
            idx_sb = cpool.tile([128, CC], I16)
            state["idx_load"] = nc.sync.dma_start(idx_sb[:], idx_d.ap()).ins.name
            hs1_keep = cpool.tile([128, NT, H], F32)
            hs2_keep = cpool.tile([128, NT, H], F32)
            zb = ZPAD // 128
            zrow = cpool.tile([128, zb, H], F32)
            nc.vector.memset(zrow[:], 0.0)

            cc1 = dpool.tile([sz.SROWS, H], F32)
            cc2 = dpool.tile([sz.SROWS, H], F32)
            nc.sync.dma_start(
                cc1[NPC: sz.SROWS, :].rearrange("(p b) f -> p b f", p=128),
                zrow[:])
            nc.sync.dma_start(
                cc2[NPC: sz.SROWS, :].rearrange("(p b) f -> p b f", p=128),
                zrow[:])

            # ---------------- gather pipeline plumbing
            # gctr counts every Pool-DMA emission (tile lane = gctr % 8);
            # prep_pos records which positions are PREPARE_ONLY preps (their
            # consumer waits get rewired to the per-queue descriptor sems).
            def gather_pair(entry, table, prep):
                (grp, lo_off, hi_off, klo_off, khi_off, slo, shi) = entry
                tiles = []
                for off, s, base in ((lo_off, slo, 0),
                                     (hi_off, shi, sz.HI_BASE)):
                    q = state["gctr"] % NQ
                    g = gpool.tile([128, GCAP, H], F32, tag="g", name="gt")
                    if prep:
                        state["prep_pos"].append(state["gctr"])
                        pi = nc.gpsimd.dma_gather(
                            out_ap=g[:, :s, :],
                            in_ap=table.ap()[base: sz.TROWS, :],
                            idxs_ap=idx_sb[:, off: off + s * 8],
                            num_idxs=s * 128, num_idxs_reg=s * 128,
                            elem_size=H, single_packet=False,
                            queue_num=q, prepare_only=True, sem=qsems[q],
                        )
                        state["qpend"][q].append(pi)
                    else:
                        nc.gpsimd.dma_gather(
                            out_ap=g[:, :s, :],
                            in_ap=table.ap()[base: sz.TROWS, :],
                            idxs_ap=idx_sb[:, off: off + s * 8],
                            num_idxs=s * 128, num_idxs_reg=s * 128,
                            elem_size=H, single_packet=False,
                            queue_num=q,
                        )
                    state["gctr"] += 1
                    tiles.append(g)
                return tiles

            def trig_wave(count):
                for q in range(NQ):
                    k = min(count, len(state["qpend"][q]))
                    if k:
                        ti = nc.gpsimd.trigger_dma(count=k, queue_num=q)
                        fired = state["qpend"][q][:k]
                        rest = state["qpend"][q][k:]
                        state["qpend"][q] = rest
                        state["trig_map"][ti.ins.name] = [
                            p.ins.name for p in fired]
                        # trigger_dma clears the whole bass pending list;
                        # push the unfired preps back so the NEXT trigger
                        # inherits their ordering deps (else the scheduler
                        # hoists it before the preps it fires).
                        nc.gpsimd._pending_untriggered_insts[q] = list(rest)

            def reduce_group(entry, tiles, epi):
                (grp, lo_off, hi_off, klo_off, khi_off, slo, shi) = entry
                g_lo, g_hi = tiles
                for t in grp:
                    psum8 = pp8.tile([128, 512], F32, tag="psum8")
                    first = True
                    for g, off, K in ((g_lo, klo_off[t], Klo[t]),
                                      (g_hi, khi_off[t], Khi[t])):
                        nmm = (K + 7) // 8
                        for j in range(nmm):
                            cnt = min(8, K - j * 8)
                            nc.tensor.matmul(
                                psum8[:, : cnt * H],
                                ident[:],
                                g[:, off + j * 8: off + j * 8 + cnt, :],
                                start=first,
                                stop=(g is g_hi and j == nmm - 1),
                            )
                            first = False
                    u = upool.tile([128, H], F32, tag="u")
                    nc.vector.tensor_reduce(
                        u[:],
                        psum8[:].rearrange("p (k f) -> p f k", k=8),
                        axis=mybir.AxisListType.X,
                        op=ADD,
                    )
                    epi(t, u)

            def layer_pipeline(table, epi):
                # First P groups are PREPARE_ONLY: their desc-gen runs under
                # phase A / the AllGather (a post-pass moves data waits onto
                # the explicit-count triggers).  Trigger wave A fires the
                # first half (fresh slots); wave B fires the rest after
                # their slots' consumers are emitted.  The bulk uses plain
                # gathers.
                n = len(plan)
                P = min(4, n)
                tiles = {}
                for gi in range(P):
                    tiles[gi] = gather_pair(plan[gi], table, prep=True)
                trig_wave(P // 2)  # one wave: all prepped slots are fresh
                for gi in range(P, n):
                    reduce_group(plan[gi - P], tiles.pop(gi - P), epi)
                    tiles[gi] = gather_pair(plan[gi], table, prep=False)
                for gi in range(n - P, n):
                    reduce_group(plan[gi], tiles.pop(gi), epi)

            # ---------------- phase A: hs1 = disp * (x @ W1) (permuted shard)
            with (
                tc.tile_pool(name="pa_sb", bufs=2) as pa,
                tc.tile_pool(name="pa_u", bufs=3) as pau,
            ):
                nch = (NT + CH - 1) // CH
                for ch in range(nch):
                    t0 = ch * CH
                    t1 = min(t0 + CH, NT)
                    cw = min((t1 - t0) * 128, NPC - t0 * 128)
                    sb_xT = pa.tile([128, 4, CH * 128], BF16, tag="sb_xT")
                    nc.sync.dma_start(
                        sb_xT[:, :, :cw],
                        xT_d.ap().rearrange("(b f) n -> f b n", b=4)
                        [:, :, t0 * 128: t0 * 128 + cw])
                    for t in range(t0, t1):
                        pn = TSZ[t]
                        co = (t - t0) * 128
                        ps_h = pps.tile([H, 128], F32, tag="ps_small")
                        for b in range(4):
                            nc.tensor.matmul(
                                ps_h[:, :pn], w1_sb[:, b, :],
                                sb_xT[:, b, co: co + pn],
                                start=(b == 0), stop=(b == 3),
                            )
                        sb_hT = pau.tile([H, 128], BF16, tag="sb_hT")
                        nc.scalar.activation(sb_hT[:, :pn], ps_h[:, :pn],
                                             ACT.Copy)
                        ps_hn = pph.tile([128, H], BF16, tag="ps_hn")
                        nc.tensor.transpose(ps_hn[:pn, :], sb_hT[:, :pn],
                                            ident_bf[:H, :H])
                        nc.vector.tensor_scalar_mul(
                            hs1_keep[:pn, t, :], ps_hn[:pn, :],
                            disp[:pn, t: t + 1])
                        nc.sync.dma_start(cc1[t * 128: t * 128 + pn, :],
                                          hs1_keep[:pn, t, :])

            # ---------------- AllGather table1
            table1 = nc.dram_tensor("table1", [sz.TROWS + 4, H], F32,
                                    kind="Internal", addr_space="Shared")
            nc.gpsimd.collective_compute(
                "AllGather", mybir.AluOpType.bypass, replica_groups=rg,
                ins=[cc1.opt()], outs=[table1.ap()[: sz.TROWS, :]],
            )

            # ---------------- layer 1 gathers + epi -> cc2
            def epi_c(t, u):
                pn = TSZ[t]
                us = upool.tile([128, H], F32, tag="us")
                nc.vector.tensor_tensor(
                    us[:], u[:], hs1_keep[:, t, :], op=ADD)
                hpre = upool.tile([128, H], F32, tag="hpre")
                nc.vector.scalar_tensor_tensor(
                    hpre[:], us[:], disp[:, t: t + 1], b1bc[:],
                    op0=MULT, op1=ADD,
                )
                h = upool.tile([128, H], F32, tag="h")
                nc.scalar.activation(h[:], hpre[:], ACT.Relu)
                nc.scalar.activation(hs2_keep[:, t, :], h[:], ACT.Copy,
                                     scale=disp[:, t: t + 1])
                nc.sync.dma_start(cc2[t * 128: t * 128 + pn, :],
                                  hs2_keep[:pn, t, :])

            layer_pipeline(table1, epi_c)

            # ---------------- AllGather table2
            table2 = nc.dram_tensor("table2", [sz.TROWS + 4, H], F32,
                                    kind="Internal", addr_space="Shared")
            nc.gpsimd.collective_compute(
                "AllGather", mybir.AluOpType.bypass, replica_groups=rg,
                ins=[cc2.opt()], outs=[table2.ap()[: sz.TROWS, :]],
            )

            # ---------------- layer 2 gathers + decoder
            def epi_e(t, u2):
                pn = TSZ[t]
                u2a = upool.tile([128, H], F32, tag="u2a")
                nc.vector.tensor_tensor(
                    u2a[:], u2[:], hs2_keep[:, t, :], op=ADD)
                u2s = epool.tile([128, H], F32, tag="u2s")
                nc.scalar.activation(u2s[:], u2a[:], ACT.Copy,
                                     scale=disp[:, t: t + 1])
                ps_uT = pps.tile([H, 128], F32, tag="ps_small")
                nc.tensor.transpose(ps_uT[:], u2s[:], ident[:])
                uT = epool.tile([H, 128], F32, tag="uT")
                nc.vector.tensor_copy(uT[:], ps_uT[:])
                ps_z = pps.tile([H, 128], F32, tag="ps_small")
                nc.tensor.matmul(ps_z[:L, :], w2_sb[:], uT[:],
                                 start=True, stop=True)
                zT = epool.tile([L, 128], F32, tag="zT")
                nc.scalar.activation(zT[:], ps_z[:L, :], ACT.Relu,
                                     bias=b2_sb[:])
                ps_d = pps.tile([H, 128], F32, tag="ps_small")
                nc.tensor.matmul(ps_d[:], wd1_sb[:], zT[:],
                                 start=True, stop=True)
                dT = epool.tile([H, 128], F32, tag="dT")
                nc.scalar.activation(dT[:], ps_d[:], ACT.Relu,
                                     bias=bd1_sb[:])
                osb = opool.tile([128, IN], F32, tag="osb")
                for gblk in range(4):
                    ps_o = ppo.tile([128, 128], F32, tag="ps_oo")
                    nc.tensor.matmul(
                        ps_o[:],
                        wd2_sb[:, gblk * 128:(gblk + 1) * 128], dT[:],
                        start=True, stop=True,
                    )
                    oT = epool.tile([128, 128], F32, tag="oT")
                    nc.scalar.activation(
                        oT[:], ps_o[:], ACT.Sigmoid,
                        bias=bd2_sb[:, gblk: gblk + 1],
                    )
                    ps_on = ppo.tile([128, 128], F32, tag="ps_oo")
                    nc.tensor.transpose(ps_on[:], oT[:], ident[:])
                    nc.vector.tensor_copy(
                        osb[:, gblk * 128:(gblk + 1) * 128], ps_on[:]
                    )
                nc.sync.dma_start(out.ap()[t * 128: t * 128 + pn, :],
                                  osb[:pn, :])

            layer_pipeline(table2, epi_e)
    return qsems, state


def _defer_prep_waits(nc, idx_load_name, trig_map):
    """Desc-gen for a PREPARE_ONLY gather reads only the index tensor; the
    table read and SBUF tile write happen at trigger+drain.  Tile pins the
    table (Collectives) and slot-WAR waits on the prep itself, which blocks
    desc-gen behind the AllGather.  Move every prep wait onto the trigger
    that fires it (per trig_map); the prep waits only for the index-tensor
    load.  The tile-inserted IncSwdgeSem pre-bumps lose their waits too
    (their lane-sem increments are order-insensitive sums)."""
    prep2trig = {}
    for tname, preps in trig_map.items():
        for p in preps:
            prep2trig[p] = tname
    for blk in nc.m.functions[0].blocks:
        cum = {}
        idx_wait = None
        moved = {}          # trigger name -> [waits]
        for inst in blk.instructions:
            si = inst.sync_info
            ups = (si.on_update or []) if si else []
            for u in ups:
                if (u.update_mode == 'sem-add-imm'
                        and (u.ant_name or '').startswith('DMAHW')):
                    cum[u.id] = cum.get(u.id, 0) + u.update_value
                    if inst.name == idx_load_name:
                        idx_wait = (u.id, u.ant_name, cum[u.id])
            tn = type(inst).__name__
            if ("GatherAnt" in tn and getattr(inst, 'gen_mode', 0) == 1):
                assert idx_wait is not None and inst.name in prep2trig
                ws = list(si.on_wait or []) if si else []
                moved.setdefault(prep2trig[inst.name], []).extend(ws)
                sid, snm, sval = idx_wait
                si.on_wait = [mybir.SyncWait(
                    sync_type='semaphore', id=sid, ant_name=snm,
                    wait_mode='sem-ge-imm', wait_value=sval, wait_reg=None)]
                inst.sync_info = si
            elif tn == "InstIncSwdgeSem" and inst._mode == "add":
                pass  # keep tile's waits on the pre-bumps
            elif "TriggerDma" in tn and inst.name in trig_map:
                mw = moved.pop(inst.name, [])
                if mw and si is not None:
                    byid = {}
                    for w in list(si.on_wait or []) + mw:
                        if (w.id not in byid
                                or w.wait_value > byid[w.id].wait_value):
                            byid[w.id] = w
                    si.on_wait = list(byid.values())
                    inst.sync_info = si
        assert not moved, f"unfired preps with moved waits: {moved}"


def _rewire_prepared_gather_waits(nc, nq, sem_by_q, prep_pos):
    """Tile routes prepared-gather completions to DMASW lane sems that no
    instruction increments at drain time (the IncSwdgeSem pre-bumps fire
    at issue time).  The descriptor-baked per-queue sems DO fire at drain
    completion; rewrite every DMASW wait that targets a PREP's tick to the
    matching per-queue sem.

    All Pool-engine DMA insts are the gathers, emitted in strict queue
    rotation: emission position p -> tile lane p%8, queue p%nq.  Lane L's
    k-th tick is position p = L + 8*(k-1).  If p is a prep, its completion
    is the m-th 16-increment of qsem[p%nq] where m = #preps with queue
    p%nq at position <= p (plain gathers keep their tile-assigned lane
    sems, which walrus bakes into their descriptors)."""
    import re
    prep_set = set(prep_pos)
    qcount = {}
    for p in sorted(prep_pos):
        qcount[p] = sum(1 for x in prep_pos if x % nq == p % nq and x <= p)
    id2lane = {}
    for blk in nc.m.functions[0].blocks:
        for inst in blk.instructions:
            si = inst.sync_info
            if not si:
                continue
            for w in (si.on_wait or []):
                m = re.match(r"DMASW(\d+)_", w.ant_name or "")
                if m:
                    id2lane[w.id] = int(m.group(1))
    for blk in nc.m.functions[0].blocks:
        for inst in blk.instructions:
            si = inst.sync_info
            if not si:
                continue
            ws = list(si.on_wait or [])
            changed = False
            for j, w in enumerate(ws):
                if w.id in id2lane and w.wait_mode == 'sem-ge-imm':
                    lane = id2lane[w.id]
                    k = w.wait_value // 16
                    p = lane + 8 * (k - 1)
                    if p not in prep_set:
                        continue
                    s = sem_by_q[p % nq]
                    ws[j] = mybir.SyncWait(
                        sync_type='semaphore', id=s.num, ant_name=s.name,
                        wait_mode='sem-ge-imm',
                        wait_value=qcount[p] * 16, wait_reg=None)
                    changed = True
            if changed:
                si.on_wait = ws
                inst.sync_info = si


def make_in_maps(sz, meta, x, W1, b1, W2, b2, Wd1, bd1, Wd2, bd2):
    b1bc = np.tile(b1[None, :], (128, 1)).astype(np.float32)
    bd2t = bd2.reshape(4, 128).T.copy().astype(np.float32)
    ident = np.eye(128, dtype=np.float32)
    in_maps = []
    for c in range(C):
        xp = x[meta["perm_nodes"][c]]
        in_maps.append({
            "xT": np.ascontiguousarray(xp.T.astype(ml_dtypes.bfloat16)),
            "idx": np.ascontiguousarray(meta["idx"][c]),
            "disp": np.ascontiguousarray(meta["disp"][c]),
            "w1": W1.astype(ml_dtypes.bfloat16), "b1bc": b1bc, "w2": W2,
            "b2": b2.reshape(L, 1).astype(np.float32),
            "wd1": Wd1, "bd1": bd1.reshape(H, 1).astype(np.float32),
            "wd2": Wd2, "bd2": bd2t, "ident": ident,
        })
    return in_maps


# ------------------------------------------------------------------- driver

def kernel(**inputs):
    x = np.ascontiguousarray(np.asarray(inputs["x"], dtype=np.float32))
    edge_index = np.asarray(inputs["edge_index"])
    args = [np.asarray(inputs[k], dtype=np.float32)
            for k in ["W1", "b1", "W2", "b2", "Wd1", "bd1", "Wd2", "bd2"]]

    sz = Sizes(x.shape[0])
    meta = _preprocess(sz, edge_index)

    nc = bacc.Bacc("TRN2", target_bir_lowering=False, debug=False,
                   num_devices=C, num_swdge_queues=NQ,
                   dynamic_dma_scratch_size=SCRATCH)
    qsems, state = build_program(nc, sz, meta)
    _defer_prep_waits(nc, state["idx_load"], state["trig_map"])
    _rewire_prepared_gather_waits(nc, NQ, qsems, state["prep_pos"])
    nc.compile()

    in_maps = make_in_maps(sz, meta, x, *args)

    trace = bool(int(os.environ.get("GCN_TRACE", "0")))
    res = run_bass_kernel_spmd(nc, in_maps, core_ids=list(range(C)),
                               trace=trace)
    global LAST_RESULTS
    LAST_RESULTS = res

    out_full = np.empty((sz.N, IN), dtype=np.float32)
    for c in range(C):
        out_full[meta["perm_nodes"][c]] = res.results[c]["out"]
    return out_full
